# revision 6
# baseline (speedup 1.0000x reference)
"""ExpHydro scan kernel for 8 Trainium2 NeuronCores (Bass/Tile).

Strategy: pure data parallelism over basins (1024 basins/core). The time
scan is sequential; per step we process all 1024 basins of a core as a
[128 partitions x 8 groups] tile. Nonlinearities (tanh/exp, one act table
set) run on ScalarE; fused custom DVE ops + GpSimd carry the arithmetic.
Inputs stream chunk-by-chunk (CH timesteps) with ping-pong DMA prefetch.

Self-contained: hardcodes shapes from the problem spec (B=8192, T=3650).
"""

import os
import sys
import math
from contextlib import ExitStack

import numpy as np

for _p in ("/opt/trn_rl_repo", "/root/.axon_site/_ro/trn_rl_repo"):
    if os.path.isdir(_p) and _p not in sys.path:
        sys.path.insert(0, _p)

import concourse.bass as bass
import concourse.tile as tile
from concourse import bacc, mybir
from concourse.bass_utils import run_bass_kernel_spmd

F32 = mybir.dt.float32
AF = mybir.ActivationFunctionType
ALU = mybir.AluOpType

B_TOT, T_TOT = 8192, 3650
NCORES = 8
BPC = B_TOT // NCORES          # 1024 basins per core
PART = 128
NG = BPC // PART               # 8 groups of 128 basins

# ----------------------------------------------------------------------------
# custom DVE ops
# ----------------------------------------------------------------------------

_CUSTOM = {}


def _register_custom_ops():
    """Register fused DVE ops at runtime (appended to dve_ops.OPS)."""
    if _CUSTOM:
        return _CUSTOM
    from concourse import dve_ops
    from concourse.dve_spec import Spec, Src0, Src1, C0, C1, One, lower
    from concourse.dve_uop import DveOpSpec

    def make(name, body, reference):
        spec = Spec(body=body, reference=reference)
        # compute the sha pins the same way DveOp.compile does
        shas = {}
        for ver in ("v3", "v4"):
            s = DveOpSpec(name=name, opcode=0, uops=lower(spec, ver=ver),
                          rd1_en=True)
            shas[ver] = s.sha(ver)
        op = dve_ops.DveOp(name, spec, subdim=False, uops_sha=shas)
        dve_ops.OPS.append(op)
        dve_ops._SUB_OPCODE_FOR_NAME[name] = (
            dve_ops._CUSTOM_DVE_ROW_BASE + len(dve_ops.OPS) - 1)
        dve_ops.CUSTOM_DVE_SPECS[name] = spec
        return op

    # (1+in0) * in1 * s0
    _CUSTOM["onep_ms"] = make(
        "ANT_EH_ONEP_MS", (One + Src0) * Src1 * C0,
        lambda in0, in1, s0, s1, imm2: (1.0 + in0) * in1 * s0)
    # (1-in0) * in1 * s0
    _CUSTOM["onem_ms"] = make(
        "ANT_EH_ONEM_MS", (One - Src0) * Src1 * C0,
        lambda in0, in1, s0, s1, imm2: (1.0 - in0) * in1 * s0)
    # (1+in0) * (in1*s0 + s1)
    _CUSTOM["onep_aff"] = make(
        "ANT_EH_ONEP_AFF", (One + Src0) * (Src1 * C0 + C1),
        lambda in0, in1, s0, s1, imm2: (1.0 + in0) * (in1 * s0 + s1))
    # (1-in0) * (in1*s0) + 1 + in0
    _CUSTOM["kcomb"] = make(
        "ANT_EH_KCOMB", (One - Src0) * (Src1 * C0) + One + Src0,
        lambda in0, in1, s0, s1, imm2: (1.0 - in0) * (in1 * s0) + 1.0 + in0)
    return _CUSTOM


# ----------------------------------------------------------------------------
# host-side scalar parameter transform (matches reference's sigmoid maps)
# ----------------------------------------------------------------------------

def host_constants(f, Smax, Qmax, Df, Tmax, Tmin):
    f32 = np.float32

    def sig(v):
        return f32(1.0 / (1.0 + math.exp(-float(v))))

    f_ = f32(sig(f) * f32(0.1))
    Smax_ = f32(sig(Smax) * f32(1400.0) + f32(100.0))
    Qmax_ = f32(sig(Qmax) * f32(50.0) + f32(10.0))
    Df_ = f32(sig(Df) * f32(5.0) + f32(0.01))
    Tmax_ = f32(sig(Tmax) * f32(3.0))
    Tmin_ = f32(sig(Tmin) * f32(-3.0))
    return f_, Smax_, Qmax_, Df_, Tmax_, Tmin_


# ----------------------------------------------------------------------------
# kernel builder
# ----------------------------------------------------------------------------

def build_nc(consts, T=T_TOT, CH=73, debug=False):
    """Build the per-core SPMD program. T must be divisible by CH."""
    f_, Smax_, Qmax_, Df_, Tmax_, Tmin_ = (np.float32(c) for c in consts)
    ops = _register_custom_ops()
    NCH = T // CH
    assert NCH * CH == T
    assert NCH % 2 == 0, "chunk count must be even (pairs)"
    NPAIR = NCH // 2

    cq4 = np.float32(Qmax_ * np.float32(0.25))
    cE4 = np.float32(math.log(float(Qmax_) / 4.0))
    ic = np.float32(np.float32(1.0) / Smax_)

    nc = bacc.Bacc("TRN2", target_bir_lowering=False, debug=debug,
                   enable_asserts=False)

    # x padded with 2*CH junk timesteps for safe prefetch overrun
    TP = T + 2 * CH
    x_d = nc.dram_tensor("x", [BPC, TP * 3], F32, kind="ExternalInput").ap()
    q_d = nc.dram_tensor("q", [BPC, T], F32, kind="ExternalOutput").ap()
    # dram views:  x[(g p) (t c)] -> [p, g, t*c],  q[(g p) t] -> [p, g, t]
    x_v = x_d.rearrange("(g p) tc -> p g tc", p=PART)
    q_v = q_d.rearrange("(g p) t -> p g t", p=PART)

    CHG = CH * NG

    with tile.TileContext(nc) as tc, ExitStack() as ctx:
        pool = ctx.enter_context(tc.tile_pool(name="main", bufs=1))

        _cmap = {}

        def cbias(val):
            v = float(np.float32(val))
            if v not in _cmap:
                ct = pool.tile([PART, 1], F32, tag=f"cb{len(_cmap)}",
                               name=f"cb{len(_cmap)}")
                nc.vector.memset(ct, v)
                _cmap[v] = ct
            return _cmap[v]

        # --- persistent tiles ---
        # state: [S2 | Z | S1] ping/pong
        tin = [pool.tile([PART, 3 * NG], F32, tag=f"tin{i}", name=f"tin{i}") for i in range(2)]
        tt = pool.tile([PART, 3 * NG], F32, tag="tt", name="tt")       # tanh outputs
        e4 = pool.tile([PART, NG], F32, tag="e4", name="e4")           # Qmax*e^{f z}/4
        # DVE temps
        tG = pool.tile([PART, NG], F32, tag="tG", name="tG")
        tH = pool.tile([PART, NG], F32, tag="tH", name="tH")
        tR = pool.tile([PART, NG], F32, tag="tR", name="tR")
        tK = pool.tile([PART, NG], F32, tag="tK", name="tK")
        tL = pool.tile([PART, NG], F32, tag="tL", name="tL")
        tW = pool.tile([PART, NG], F32, tag="tW", name="tW")
        tX = pool.tile([PART, NG], F32, tag="tX", name="tX")
        tY = pool.tile([PART, NG], F32, tag="tY", name="tY")
        # POOL temps
        tMh = pool.tile([PART, NG], F32, tag="tMh", name="tMh")
        tMg = pool.tile([PART, NG], F32, tag="tMg", name="tMg")
        tM = pool.tile([PART, NG], F32, tag="tM", name="tM")
        tMM = pool.tile([PART, NG], F32, tag="tMM", name="tMM")
        tD1 = pool.tile([PART, NG], F32, tag="tD1", name="tD1")

        # raw input chunks (ping/pong), group-major [p, g, t, c]
        raw = [pool.tile([PART, NG * CH * 3], F32, tag=f"raw{i}", name=f"raw{i}")
               for i in range(2)]
        # derived per-chunk arrays, group-major [p, g, t]
        der = []
        for i in range(2):
            d = {}
            for nm in ("pr", "ps", "dft", "pet4", "st3h"):
                d[nm] = pool.tile([PART, CHG], F32, tag=f"{nm}{i}", name=f"{nm}{i}")
            der.append(d)
        th = pool.tile([PART, CHG], F32, tag="th", name="th")  # bulk tanh scratch
        qc = [pool.tile([PART, CHG], F32, tag=f"qc{i}", name=f"qc{i}") for i in range(2)]

        def raw_view(i, c):
            # [p, g, t, c=3] -> channel-sliced [p, g, t]
            return raw[i].rearrange("p (g t c) -> p g t c", g=NG, t=CH)[
                :, :, :, c]

        def der_view(i, nm):
            return der[i][nm].rearrange("p (g t) -> p g t", g=NG)

        def bulk(i):
            """Derive chunk arrays from raw[i] into der[i]."""
            P = raw_view(i, 0)
            Tt = raw_view(i, 1)
            Pet = raw_view(i, 2)
            d = der[i]
            thv = th.rearrange("p (g t) -> p g t", g=NG)
            # st3h = (tanh(5T - 5Tmax)+1)/4
            nc.scalar.activation(thv, Tt, AF.Tanh,
                                 bias=cbias(-5.0 * Tmax_), scale=5.0)
            nc.scalar.activation(der_view(i, "st3h"), thv, AF.Identity,
                                 bias=cbias(0.25), scale=0.25)
            # Pr = (tanh(5T - 5Tmin)+1) * (P*0.5);  Ps = P - Pr
            nc.scalar.activation(thv, Tt, AF.Tanh,
                                 bias=cbias(-5.0 * Tmin_), scale=5.0)
            nc.vector._custom_dve(ops["onep_ms"], out=der_view(i, "pr"),
                                  in0=thv, in1=P, s0=0.5)
            nc.vector.tensor_tensor(der_view(i, "ps"), P, der_view(i, "pr"),
                                    ALU.subtract)
            # DfT = (T - Tmax)*Df   (ACT affine)
            nc.scalar.activation(der_view(i, "dft"), Tt, AF.Identity,
                                 bias=cbias(-Df_ * Tmax_), scale=float(Df_))
            # Pet4 = Pet * 0.25
            nc.scalar.activation(der_view(i, "pet4"), Pet, AF.Copy,
                                 bias=0.0, scale=0.25)

        def inner(i, pt0, qi):
            """Run CH steps using der[i]; state parity starts at pt0."""
            d = der[i]
            qcv = qc[qi].rearrange("p (g t) -> p g t", g=NG)
            for t in range(CH):
                cur = tin[(pt0 + t) % 2]
                nxt = tin[(pt0 + t + 1) % 2]
                S2 = cur[:, 0:NG]
                Z = cur[:, NG:2 * NG]
                S1 = cur[:, 2 * NG:3 * NG]
                T1 = tt[:, 0:NG]
                T2 = tt[:, NG:2 * NG]
                T4 = tt[:, 2 * NG:3 * NG]
                prt = der_view(i, "pr")[:, :, t]
                pst = der_view(i, "ps")[:, :, t]
                dftt = der_view(i, "dft")[:, :, t]
                pet4t = der_view(i, "pet4")[:, :, t]
                st3ht = der_view(i, "st3h")[:, :, t]

                # ACT: tanh batch + exp (same table set)
                nc.scalar.activation(tt, cur, AF.Tanh, bias=cbias(0.0),
                                     scale=5.0)
                nc.scalar.activation(e4, Z, AF.Exp, bias=cbias(cE4),
                                     scale=float(f_))

                # melt + S1 chain: min/stt on DVE, pure arith on POOL
                nc.vector.tensor_tensor(tMh, S1, dftt, ALU.min)
                nc.vector._custom_dve(ops["onep_ms"], out=tMg, in0=T4,
                                      in1=st3ht, s0=1.0)
                nc.gpsimd.tensor_tensor(tM, tMh, tMg, ALU.mult)
                nc.gpsimd.tensor_tensor(tMM, tM, prt, ALU.add)
                nc.gpsimd.tensor_tensor(tD1, pst, tM, ALU.subtract)
                nc.gpsimd.tensor_tensor(nxt[:, 2 * NG:3 * NG], S1, tD1,
                                        ALU.add)

                # DVE: Q / ET / state update
                nc.vector._custom_dve(ops["onep_aff"], out=tH, in0=T2,
                                      in1=Z, s0=0.25, s1=float(cq4))
                nc.vector._custom_dve(ops["kcomb"], out=tK, in0=T2,
                                      in1=S2, s0=float(ic))
                nc.vector._custom_dve(ops["onem_ms"], out=tG, in0=T2,
                                      in1=e4, s0=1.0)
                nc.vector.tensor_tensor(tR, tG, tH, ALU.add)
                nc.vector._custom_dve(ops["onep_ms"], out=qcv[:, :, t],
                                      in0=T1, in1=tR, s0=1.0)
                nc.vector.tensor_tensor(tL, tK, pet4t, ALU.mult)
                nc.vector.tensor_tensor(tW, tL, tR, ALU.add)
                nc.vector._custom_dve(ops["onep_ms"], out=tX, in0=T1,
                                      in1=tW, s0=1.0)
                nc.gpsimd.tensor_tensor(tY, tMM, tX, ALU.subtract)
                nc.vector.tensor_tensor(nxt[:, 0:NG], S2, tY, ALU.add)
                nc.vector.tensor_scalar(nxt[:, NG:2 * NG], nxt[:, 0:NG],
                                        float(Smax_), None, ALU.subtract)

        def dma_in(i, coff):
            """Load raw chunk; coff = element column offset (python int or
            register value expr)."""
            src = x_v[:, :, bass.ds(coff, CH * 3)]
            nc.sync.dma_start(out=raw[i].rearrange(
                "p (g tc) -> p g tc", g=NG), in_=src)

        def dma_out(qi, toff):
            dst = q_v[:, :, bass.ds(toff, CH)]
            nc.sync.dma_start(out=dst, in_=qc[qi].rearrange(
                "p (g t) -> p g t", g=NG))

        # --- init state ---
        nc.vector.memset(tin[0][:, 0:NG], 0.0)
        nc.vector.memset(tin[0][:, NG:2 * NG], float(-Smax_))
        nc.vector.memset(tin[0][:, 2 * NG:3 * NG], 0.0)

        # --- prologue: chunk 0 into raw0/der0, chunk 1 into raw1 ---
        dma_in(0, 0)
        bulk(0)
        dma_in(1, CH * 3)

        # --- main loop over chunk pairs ---
        # invariant at iteration p (chunks c0=2p, c1=2p+1):
        #   raw0/der0 hold chunk c0 (bulk done), raw1 holds chunk c1 (raw)
        def body(iv):
            c0e = iv * (2 * CH * 3)          # element offset of chunk 2p
            bulk(1)                          # chunk 2p+1 derive
            inner(0, 0, 0)                   # chunk 2p  (parity: 2p*CH even)
            dma_out(0, iv * 2 * CH)
            dma_in(0, c0e + 2 * CH * 3)      # prefetch chunk 2p+2
            inner(1, CH % 2, 1)              # chunk 2p+1
            dma_out(1, iv * 2 * CH + CH)
            bulk(0)                          # chunk 2p+2 derive
            dma_in(1, c0e + 3 * CH * 3)      # prefetch chunk 2p+3 (may
            #                                  overrun into the junk pad)

        if NPAIR == 1:
            body(0)
        else:
            with tc.For_i(0, NPAIR, 1) as iv:
                body(iv)

    nc.compile()
    return nc


# ----------------------------------------------------------------------------
# public entry point
# ----------------------------------------------------------------------------

_NC_CACHE = {}
TRACE = False
LAST_EXEC_NS = None


def _get_nc(consts):
    key = tuple(float(c) for c in consts)
    if key not in _NC_CACHE:
        _NC_CACHE[key] = build_nc(consts)
    return _NC_CACHE[key]


def kernel(x, f, Smax, Qmax, Df, Tmax, Tmin):
    x = np.asarray(x, dtype=np.float32)
    assert x.shape == (B_TOT, T_TOT, 3), x.shape
    consts = host_constants(float(np.asarray(f)), float(np.asarray(Smax)),
                            float(np.asarray(Qmax)), float(np.asarray(Df)),
                            float(np.asarray(Tmax)), float(np.asarray(Tmin)))
    nc = _get_nc(consts)

    CH = 73
    pad = np.zeros((BPC, 2 * CH * 3), np.float32)
    in_maps = []
    for c in range(NCORES):
        xc = np.ascontiguousarray(
            x[c * BPC:(c + 1) * BPC].reshape(BPC, T_TOT * 3))
        in_maps.append({"x": np.concatenate([xc, pad], axis=1)})

    rr = run_bass_kernel_spmd(nc, in_maps, core_ids=list(range(NCORES)),
                              trace=TRACE)
    global LAST_EXEC_NS
    LAST_EXEC_NS = rr.exec_time_ns
    out = np.concatenate([rr.results[c]["q"] for c in range(NCORES)], axis=0)
    return out.astype(np.float32)


# revision 9
# speedup vs baseline: 1.2177x; 1.2177x over previous
"""ExpHydro scan kernel for 8 Trainium2 NeuronCores (Bass/Tile).

Strategy: pure data parallelism over basins (1024 basins/core). The time
scan is sequential; per step we process all 1024 basins of a core as a
[128 partitions x 8 groups] tile. Nonlinearities (tanh/exp, one act table
set) run on ScalarE; fused custom DVE ops (incl. two paged 2-in-1 ops)
carry the arithmetic; GpSimd runs the melt/S1 chain and the Q output mul.
Inputs stream chunk-by-chunk (CH timesteps) with ping-pong DMA prefetch.

Self-contained: hardcodes shapes from the problem spec (B=8192, T=3650).
"""

import os
import sys
import math
from contextlib import ExitStack

import numpy as np

for _p in ("/opt/trn_rl_repo", "/root/.axon_site/_ro/trn_rl_repo"):
    if os.path.isdir(_p) and _p not in sys.path:
        sys.path.insert(0, _p)

import concourse.bass as bass
import concourse.tile as tile
from concourse import bacc, mybir
from concourse.bass_utils import run_bass_kernel_spmd

F32 = mybir.dt.float32
AF = mybir.ActivationFunctionType
ALU = mybir.AluOpType

B_TOT, T_TOT = 8192, 3650
NCORES = 8
BPC = B_TOT // NCORES          # 1024 basins per core
PART = 128
NG = BPC // PART               # 8 groups of 128 basins

# ----------------------------------------------------------------------------
# custom DVE ops
# ----------------------------------------------------------------------------

_CUSTOM = {}


def _register_custom_ops():
    """Register fused DVE ops at runtime (appended to dve_ops.OPS)."""
    if _CUSTOM:
        return _CUSTOM
    from concourse import dve_ops
    from concourse.dve_spec import (Spec, Src0, Src1, C0, C1, One, Zero,
                                    SubIdx, eq, minn, select, lower)
    from concourse.dve_uop import DveOpSpec

    def make(name, body, reference, subdim=False):
        spec = Spec(body=body, reference=reference)
        shas = {}
        for ver in ("v3", "v4"):
            s = DveOpSpec(name=name, opcode=0, uops=lower(spec, ver=ver),
                          rd1_en=True)
            shas[ver] = s.sha(ver)
        op = dve_ops.DveOp(name, spec, subdim=subdim, uops_sha=shas)
        dve_ops.OPS.append(op)
        dve_ops._SUB_OPCODE_FOR_NAME[name] = (
            dve_ops._CUSTOM_DVE_ROW_BASE + len(dve_ops.OPS) - 1)
        dve_ops.CUSTOM_DVE_SPECS[name] = spec
        return op

    # (1+in0) * in1 * s0
    _CUSTOM["onep_ms"] = make(
        "ANT_EH_ONEP_MS", (One + Src0) * Src1 * C0,
        lambda in0, in1, s0, s1, imm2: (1.0 + in0) * in1 * s0)
    # (1-in0) * in1 * s0
    _CUSTOM["onem_ms"] = make(
        "ANT_EH_ONEM_MS", (One - Src0) * Src1 * C0,
        lambda in0, in1, s0, s1, imm2: (1.0 - in0) * in1 * s0)
    # (1+in0) * (in1*s0 + s1)
    _CUSTOM["onep_aff"] = make(
        "ANT_EH_ONEP_AFF", (One + Src0) * (Src1 * C0 + C1),
        lambda in0, in1, s0, s1, imm2: (1.0 + in0) * (in1 * s0 + s1))
    # (1-in0) * (in1*s0) + 1 + in0
    _CUSTOM["kcomb"] = make(
        "ANT_EH_KCOMB", (One - Src0) * (Src1 * C0) + One + Src0,
        lambda in0, in1, s0, s1, imm2: (1.0 - in0) * (in1 * s0) + 1.0 + in0)
    # paged [P,2,N]: page0 = min(in0,in1); page1 = (1+in0)*in1
    _CUSTOM["mhmg"] = make(
        "ANT_EH_MHMG",
        select(eq(SubIdx, Zero), minn(Src0, Src1), (One + Src0) * Src1),
        lambda in0, in1, s0, s1, imm2: np.stack(
            [np.minimum(in0[:, 0], in1[:, 0]),
             (1.0 + in0[:, 1]) * in1[:, 1]], axis=1),
        subdim=True)
    # paged [P,2,N]: page0 = in0*in1; page1 = in0+in1
    _CUSTOM["lrop"] = make(
        "ANT_EH_LROP",
        select(eq(SubIdx, Zero), Src0 * Src1, Src0 + Src1),
        lambda in0, in1, s0, s1, imm2: np.stack(
            [in0[:, 0] * in1[:, 0], in0[:, 1] + in1[:, 1]], axis=1),
        subdim=True)
    return _CUSTOM


# ----------------------------------------------------------------------------
# host-side scalar parameter transform (matches reference's sigmoid maps)
# ----------------------------------------------------------------------------

def host_constants(f, Smax, Qmax, Df, Tmax, Tmin):
    f32 = np.float32

    def sig(v):
        return f32(1.0 / (1.0 + math.exp(-float(v))))

    f_ = f32(sig(f) * f32(0.1))
    Smax_ = f32(sig(Smax) * f32(1400.0) + f32(100.0))
    Qmax_ = f32(sig(Qmax) * f32(50.0) + f32(10.0))
    Df_ = f32(sig(Df) * f32(5.0) + f32(0.01))
    Tmax_ = f32(sig(Tmax) * f32(3.0))
    Tmin_ = f32(sig(Tmin) * f32(-3.0))
    return f_, Smax_, Qmax_, Df_, Tmax_, Tmin_


# ----------------------------------------------------------------------------
# kernel builder
# ----------------------------------------------------------------------------

def build_nc(consts, T=T_TOT, CH=73, debug=False):
    """Build the per-core SPMD program. T must be divisible by CH."""
    f_, Smax_, Qmax_, Df_, Tmax_, Tmin_ = (np.float32(c) for c in consts)
    ops = _register_custom_ops()
    NCH = T // CH
    assert NCH * CH == T
    assert NCH % 2 == 0, "chunk count must be even (pairs)"
    NPAIR = NCH // 2

    # exp bias: ln(Qmax/4) - f*Smax  (exp arg = f*S2 + bias, single-round fma)
    cE4 = np.float32(math.log(float(Qmax_) / 4.0) - float(f_) * float(Smax_))
    # H constant: (Qmax - Smax)/4
    cq4 = np.float32((float(Qmax_) - float(Smax_)) / 4.0)
    ic = np.float32(np.float32(1.0) / Smax_)

    nc = bacc.Bacc("TRN2", target_bir_lowering=False, debug=debug,
                   enable_asserts=False)

    # x padded with 2*CH junk timesteps for safe prefetch overrun
    TP = T + 2 * CH
    x_d = nc.dram_tensor("x", [BPC, TP * 3], F32, kind="ExternalInput").ap()
    q_d = nc.dram_tensor("q", [BPC, T], F32, kind="ExternalOutput").ap()
    x_v = x_d.rearrange("(g p) tc -> p g tc", p=PART)
    q_v = q_d.rearrange("(g p) t -> p g t", p=PART)

    CHG = CH * NG

    with tile.TileContext(nc) as tc, ExitStack() as ctx:
        pool = ctx.enter_context(tc.tile_pool(name="main", bufs=1))

        _cmap = {}

        def cbias(val):
            v = float(np.float32(val))
            if v not in _cmap:
                ct = pool.tile([PART, 1], F32, tag=f"cb{len(_cmap)}",
                               name=f"cb{len(_cmap)}")
                nc.vector.memset(ct, v)
                _cmap[v] = ct
            return _cmap[v]

        # --- persistent tiles ---
        # combined state+act tile (ping/pong by step parity):
        # [S2 | S1 | T1 | T4 | T2 | E4] each NG cols
        sb = [pool.tile([PART, 6 * NG], F32, tag=f"sb{i}", name=f"sb{i}")
              for i in range(2)]
        kg = pool.tile([PART, 2 * NG], F32, tag="kg", name="kg")   # [K | G]
        lr = pool.tile([PART, 2 * NG], F32, tag="lr", name="lr")   # [L | R]
        mm2 = pool.tile([PART, 2 * NG], F32, tag="mm2", name="mm2")  # [mA|mB]
        tW = pool.tile([PART, NG], F32, tag="tW", name="tW")
        tX = pool.tile([PART, NG], F32, tag="tX", name="tX")
        tY = pool.tile([PART, NG], F32, tag="tY", name="tY")
        tM = pool.tile([PART, NG], F32, tag="tM", name="tM")
        tMM = pool.tile([PART, NG], F32, tag="tMM", name="tMM")
        tD1 = pool.tile([PART, NG], F32, tag="tD1", name="tD1")
        tq1 = pool.tile([PART, NG], F32, tag="tq1", name="tq1")

        # raw input chunks (ping/pong), group-major [p, g, t, c]
        raw = [pool.tile([PART, NG * CH * 3], F32, tag=f"raw{i}",
                         name=f"raw{i}") for i in range(2)]
        # derived per-chunk arrays:
        #  dfst = [DfT | st3h], pet2 = [Pet/4 | H-scratch], pr, ps
        der = []
        for i in range(2):
            d = {
                "dfst": pool.tile([PART, 2 * CHG], F32, tag=f"dfst{i}",
                                  name=f"dfst{i}"),
                "pet2": pool.tile([PART, 2 * CHG], F32, tag=f"pet2{i}",
                                  name=f"pet2{i}"),
                "pr": pool.tile([PART, CHG], F32, tag=f"pr{i}",
                                name=f"pr{i}"),
                "ps": pool.tile([PART, CHG], F32, tag=f"ps{i}",
                                name=f"ps{i}"),
            }
            der.append(d)
        th = pool.tile([PART, CHG], F32, tag="th", name="th")
        qc = [pool.tile([PART, CHG], F32, tag=f"qc{i}", name=f"qc{i}")
              for i in range(2)]

        def raw_view(i, c):
            return raw[i].rearrange("p (g t c) -> p g t c", g=NG, t=CH)[
                :, :, :, c]

        def gt(ap):
            """[p, (g t)] -> [p, g, t] view."""
            return ap.rearrange("p (g t) -> p g t", g=NG)

        def bulk(i):
            """Derive chunk arrays from raw[i] into der[i]."""
            P = raw_view(i, 0)
            Tt = raw_view(i, 1)
            Pet = raw_view(i, 2)
            d = der[i]
            thv = gt(th)
            dfst4 = d["dfst"].rearrange("p (s g t) -> p s g t", s=2, g=NG)
            pet24 = d["pet2"].rearrange("p (s g t) -> p s g t", s=2, g=NG)
            # DfT = (T - Tmax)*Df
            nc.scalar.activation(dfst4[:, 0], Tt, AF.Identity,
                                 bias=cbias(-Df_ * Tmax_), scale=float(Df_))
            # st3h = (tanh(5T - 5Tmax)+1)/4
            nc.scalar.activation(thv, Tt, AF.Tanh,
                                 bias=cbias(-5.0 * Tmax_), scale=5.0)
            nc.scalar.activation(dfst4[:, 1], thv, AF.Identity,
                                 bias=cbias(0.25), scale=0.25)
            # Pet4 = Pet * 0.25
            nc.scalar.activation(pet24[:, 0], Pet, AF.Copy,
                                 bias=0.0, scale=0.25)
            # Pr = (tanh(5T - 5Tmin)+1) * (P*0.5);  Ps = P - Pr
            nc.scalar.activation(thv, Tt, AF.Tanh,
                                 bias=cbias(-5.0 * Tmin_), scale=5.0)
            nc.vector._custom_dve(ops["onep_ms"], out=gt(d["pr"]),
                                  in0=thv, in1=P, s0=0.5)
            nc.vector.tensor_tensor(gt(d["ps"]), P, gt(d["pr"]),
                                    ALU.subtract)

        def inner(i, pt0, qi):
            """Run CH steps using der[i]; state parity starts at pt0."""
            d = der[i]
            qcv = gt(qc[qi])
            dfst4 = d["dfst"].rearrange("p (s g t) -> p s g t", s=2, g=NG)
            pet24 = d["pet2"].rearrange("p (s g t) -> p s g t", s=2, g=NG)
            prv, psv = gt(d["pr"]), gt(d["ps"])
            kg3 = kg.rearrange("p (s n) -> p s n", s=2)
            lr3 = lr.rearrange("p (s n) -> p s n", s=2)
            mm23 = mm2.rearrange("p (s n) -> p s n", s=2)
            for t in range(CH):
                cur = sb[(pt0 + t) % 2]
                nxt = sb[(pt0 + t + 1) % 2]
                cur6 = cur.rearrange("p (c n) -> p c n", c=6)
                S2 = cur[:, 0:NG]
                S1 = cur[:, NG:2 * NG]
                T1 = cur[:, 2 * NG:3 * NG]
                T4 = cur[:, 3 * NG:4 * NG]
                T2 = cur[:, 4 * NG:5 * NG]
                E4 = cur[:, 5 * NG:6 * NG]
                # [S1 | T4] = slots 1 and 3: page-stride 2*NG view
                s1t4 = cur.rearrange("p (a b n) -> p b a n", a=3, b=2)[:, 1][:, 0:2]
                prt, pst = prv[:, :, t], psv[:, :, t]
                dfstt = dfst4[:, :, :, t]
                pet2t = pet24[:, :, :, t]
                hsl = pet24[:, 1, :, t]                  # H scratch slice

                # ACT (order: tanh16 -> T2 -> exp; all exp_and_others)
                nc.scalar.activation(cur[:, 2 * NG:4 * NG], cur[:, 0:2 * NG],
                                     AF.Tanh, bias=cbias(0.0), scale=5.0)
                nc.scalar.activation(T2, S2, AF.Tanh,
                                     bias=cbias(-5.0 * Smax_), scale=5.0)
                nc.scalar.activation(E4, S2, AF.Exp, bias=cbias(cE4),
                                     scale=float(f_))

                # DVE: paged melt op first (feeds POOL's M chain)
                nc.vector._custom_dve(ops["mhmg"], out=mm23, in0=s1t4,
                                      in1=dfstt)
                # POOL: melt/S1 chain
                nc.gpsimd.tensor_tensor(tM, mm2[:, 0:NG], mm2[:, NG:2 * NG],
                                        ALU.mult)
                nc.gpsimd.tensor_tensor(tMM, tM, prt, ALU.add)
                nc.gpsimd.tensor_tensor(tD1, pst, tM, ALU.subtract)
                nc.gpsimd.tensor_tensor(nxt[:, NG:2 * NG], S1, tD1, ALU.add)
                nc.gpsimd.tensor_scalar_add(tq1, T1, 1.0)
                # DVE stream
                nc.vector._custom_dve(ops["onep_aff"], out=hsl, in0=T2,
                                      in1=S2, s0=0.25, s1=float(cq4))
                nc.vector._custom_dve(ops["kcomb"], out=kg[:, 0:NG], in0=T2,
                                      in1=S2, s0=float(ic))
                nc.vector._custom_dve(ops["onem_ms"], out=kg[:, NG:2 * NG],
                                      in0=T2, in1=E4, s0=1.0)
                nc.vector._custom_dve(ops["lrop"], out=lr3, in0=kg3,
                                      in1=pet2t)
                nc.gpsimd.tensor_tensor(qcv[:, :, t], tq1, lr[:, NG:2 * NG],
                                        ALU.mult)
                nc.vector.tensor_tensor(tW, lr[:, 0:NG], lr[:, NG:2 * NG],
                                        ALU.add)
                nc.vector._custom_dve(ops["onep_ms"], out=tX, in0=T1,
                                      in1=tW, s0=1.0)
                nc.vector.tensor_tensor(tY, tMM, tX, ALU.subtract)
                nc.vector.tensor_tensor(nxt[:, 0:NG], S2, tY, ALU.add)

        def dma_in(i, coff):
            src = x_v[:, :, bass.ds(coff, CH * 3)]
            nc.sync.dma_start(out=raw[i].rearrange(
                "p (g tc) -> p g tc", g=NG), in_=src)

        def dma_out(qi, toff):
            dst = q_v[:, :, bass.ds(toff, CH)]
            nc.sync.dma_start(out=dst, in_=gt(qc[qi]))

        # --- init state ---
        nc.vector.memset(sb[0][:, 0:NG], 0.0)
        nc.vector.memset(sb[0][:, NG:2 * NG], 0.0)

        # --- prologue: chunk 0 into raw0/der0, chunk 1 into raw1 ---
        dma_in(0, 0)
        bulk(0)
        dma_in(1, CH * 3)

        def body(iv):
            c0e = iv * (2 * CH * 3)
            bulk(1)
            inner(0, 0, 0)
            dma_out(0, iv * 2 * CH)
            dma_in(0, c0e + 2 * CH * 3)
            inner(1, CH % 2, 1)
            dma_out(1, iv * 2 * CH + CH)
            bulk(0)
            dma_in(1, c0e + 3 * CH * 3)

        if NPAIR == 1:
            body(0)
        else:
            with tc.For_i(0, NPAIR, 1) as iv:
                body(iv)

    nc.compile()
    return nc


# ----------------------------------------------------------------------------
# public entry point
# ----------------------------------------------------------------------------

_NC_CACHE = {}
TRACE = False
LAST_EXEC_NS = None


def _get_nc(consts):
    key = tuple(float(c) for c in consts)
    if key not in _NC_CACHE:
        _NC_CACHE[key] = build_nc(consts)
    return _NC_CACHE[key]


def kernel(x, f, Smax, Qmax, Df, Tmax, Tmin):
    x = np.asarray(x, dtype=np.float32)
    assert x.shape == (B_TOT, T_TOT, 3), x.shape
    consts = host_constants(float(np.asarray(f)), float(np.asarray(Smax)),
                            float(np.asarray(Qmax)), float(np.asarray(Df)),
                            float(np.asarray(Tmax)), float(np.asarray(Tmin)))
    nc = _get_nc(consts)

    CH = 73
    pad = np.zeros((BPC, 2 * CH * 3), np.float32)
    in_maps = []
    for c in range(NCORES):
        xc = np.ascontiguousarray(
            x[c * BPC:(c + 1) * BPC].reshape(BPC, T_TOT * 3))
        in_maps.append({"x": np.concatenate([xc, pad], axis=1)})

    rr = run_bass_kernel_spmd(nc, in_maps, core_ids=list(range(NCORES)),
                              trace=TRACE)
    global LAST_EXEC_NS
    LAST_EXEC_NS = rr.exec_time_ns
    out = np.concatenate([rr.results[c]["q"] for c in range(NCORES)], axis=0)
    return out.astype(np.float32)
